# revision 1
# baseline (speedup 1.0000x reference)
"""Trainium2 Bass kernel for spatial attention (GroupNorm + QKV + softmax attention
+ output projection + residual), distributed over 8 NeuronCores.

Sharding: core = 2*b + hp handles image b (of 4) and head pair hp (heads 2hp, 2hp+1).
Each core computes GroupNorm(x[b]), its heads' q/k/v, full spatial attention for its
two heads, and a partial output projection (out_w columns for its heads). Core pairs'
partial outputs are summed on the host (hp==0 core carries the residual + bias).
"""

import numpy as np

import concourse.bass as bass
import concourse.bacc as bacc
import concourse.tile as tile
from concourse import mybir
from concourse import bass_utils
from concourse.alu_op_type import AluOpType

B, C, H, W = 4, 256, 48, 48
HW = H * W  # 2304
NH, HD = 4, 64
G, GC = 16, 16  # 16 groups x 16 channels
EPS = 1e-5
NCORES = 8
JC = 128  # j (key spatial) chunk
NJ = HW // JC  # 18
IBLKS = [(0, 512), (512, 1024), (1024, 1536), (1536, 2048), (2048, 2304)]
HALF = HW // 2  # 1152

F32 = mybir.dt.float32
F32R = mybir.dt.float32r
BF16 = mybir.dt.bfloat16
AX = mybir.AxisListType.X
AF = mybir.ActivationFunctionType
OP = AluOpType


def _nchunks(size, step=512):
    # PSUM-bank-aligned chunks: a matmul output may not cross a 512-fp32 bank boundary
    return [(a, min(a + step, size)) for a in range(0, size, step)]


def _build(mm_dt=F32R):
    nc = bacc.Bacc("TRN2", target_bir_lowering=False, debug=False, enable_asserts=False)

    def cast(ap):
        return ap

    x_d = nc.dram_tensor("x", [C, HW], F32, kind="ExternalInput").ap()
    res_d = nc.dram_tensor("res", [C, HW], F32, kind="ExternalInput").ap()
    wq_d = nc.dram_tensor("wq", [C, 2 * HD], F32, kind="ExternalInput").ap()
    wk_d = nc.dram_tensor("wk", [C, 2 * HD], F32, kind="ExternalInput").ap()
    wv_d = nc.dram_tensor("wv", [C, 2 * HD], F32, kind="ExternalInput").ap()
    wo_d = nc.dram_tensor("wo", [2 * HD, C], F32, kind="ExternalInput").ap()
    gnp_d = nc.dram_tensor("gnp", [C, 2], F32, kind="ExternalInput").ap()
    gind_d = nc.dram_tensor("gind", [128, 32], F32, kind="ExternalInput").ap()
    gbc_d = nc.dram_tensor("gbc", [16, C], F32, kind="ExternalInput").ap()
    y_d = nc.dram_tensor("y", [C, HW], F32, kind="ExternalOutput").ap()

    with tile.TileContext(nc) as tc:
        with (
            tc.tile_pool(name="consts", bufs=1) as consts,
            tc.tile_pool(name="big", bufs=1) as big,
            tc.tile_pool(name="small", bufs=4) as small,
            tc.tile_pool(name="pt", bufs=4) as ptp,
        ):
            # ---- constant / weight loads ----
            gind_sb = consts.tile([128, 32], F32, tag="gind", name="gind")
            nc.sync.dma_start(gind_sb[:], gind_d[:])
            gbc_sb = consts.tile([16, C], F32, tag="gbc", name="gbc")
            nc.sync.dma_start(gbc_sb[:], gbc_d[:])
            w_sb = {}
            for name, d in (("wq", wq_d), ("wk", wk_d), ("wv", wv_d)):
                for kc in range(2):
                    tf = consts.tile([128, 2 * HD], F32, tag=f"{name}{kc}f", name=f"{name}{kc}f")
                    nc.sync.dma_start(tf[:], d[kc * 128 : (kc + 1) * 128, :])
                    t = consts.tile([128, 2 * HD], mm_dt, tag=f"{name}{kc}", name=f"{name}{kc}")
                    nc.vector.tensor_copy(t[:], tf[:])
                    w_sb[name, kc] = t
            wof = consts.tile([128, C], F32, tag="wof", name="wof")
            nc.sync.dma_start(wof[:], wo_d[:])
            wo_sb = consts.tile([128, C], mm_dt, tag="wo", name="wo")
            nc.vector.tensor_copy(wo_sb[:], wof[:])
            gnp_sb = []
            for ct in range(2):
                t = consts.tile([128, 2], F32, tag=f"gnp{ct}", name=f"gnp{ct}")
                nc.sync.dma_start(t[:], gnp_d[ct * 128 : (ct + 1) * 128, :])
                gnp_sb.append(t)

            x_sb, xn_sb, res_sb = [], [], []
            for ct in range(2):
                t = big.tile([128, HW], F32, tag=f"x{ct}", name=f"x{ct}")
                nc.sync.dma_start(t[:], x_d[ct * 128 : (ct + 1) * 128, :])
                x_sb.append(t)
                xn_sb.append(big.tile([128, HW], mm_dt, tag=f"xn{ct}", name=f"xn{ct}"))
                r = big.tile([128, HW], F32, tag=f"res{ct}", name=f"res{ct}")
                nc.sync.dma_start(r[:], res_d[ct * 128 : (ct + 1) * 128, :])
                res_sb.append(r)

            # ---- GroupNorm ----
            # per-channel sums / sum-of-squares -> stats[:, (s0,q0,s1,q1)]
            stats = small.tile([128, 4], F32, tag="stats", name="stats")
            for ct in range(2):
                nc.vector.reduce_sum(stats[:, 2 * ct : 2 * ct + 1], x_sb[ct][:], axis=AX)
                nc.vector.scalar_tensor_tensor(
                    xn_sb[ct][:], x_sb[ct][:], 1.0, x_sb[ct][:],
                    op0=OP.mult, op1=OP.mult,
                    accum_out=stats[:, 2 * ct + 1 : 2 * ct + 2],
                )
            with tc.tile_pool(name="ps_gn", bufs=2, space=bass.MemorySpace.PSUM) as ps_gn:
                # accumulate both channel tiles' per-group (sum, sumsq) into [16, 2]
                g_ps = ps_gn.tile([16, 2], F32, tag="g", name="g")
                nc.tensor.matmul(g_ps[:], gind_sb[:, 0:16], stats[:, 0:2], start=True, stop=False)
                nc.tensor.matmul(g_ps[:], gind_sb[:, 16:32], stats[:, 2:4], start=False, stop=True)
                mall = small.tile([16, 2], F32, tag="mall", name="mall")
                nc.vector.tensor_scalar_mul(mall[:], g_ps[:], 1.0 / (GC * HW))
                msq = small.tile([16, 1], F32, tag="msq", name="msq")
                nc.vector.tensor_tensor(msq[:], mall[:, 0:1], mall[:, 0:1], op=OP.mult)
                ve = small.tile([16, 1], F32, tag="ve", name="ve")
                nc.vector.tensor_tensor(ve[:], mall[:, 1:2], msq[:], op=OP.subtract)
                ve2 = small.tile([16, 1], F32, tag="ve2", name="ve2")
                nc.vector.tensor_scalar_add(ve2[:], ve[:], EPS)
                sq = small.tile([16, 1], F32, tag="sq", name="sq")
                nc.scalar.activation(sq[:], ve2[:], AF.Sqrt)
                r0 = small.tile([16, 1], F32, tag="r0", name="r0")
                nc.vector.reciprocal(r0[:], sq[:])
                # sqrt LUT is loose; one Newton step: r = r0 * (1.5 - 0.5 * v * r0^2)
                t1 = small.tile([16, 1], F32, tag="t1", name="t1")
                nc.vector.tensor_tensor(t1[:], r0[:], r0[:], op=OP.mult)
                t2 = small.tile([16, 1], F32, tag="t2", name="t2")
                nc.vector.tensor_tensor(t2[:], ve2[:], t1[:], op=OP.mult)
                t3 = small.tile([16, 1], F32, tag="t3", name="t3")
                nc.vector.tensor_scalar(t3[:], t2[:], -0.5, 1.5, op0=OP.mult, op1=OP.add)
                # gvals [16, 2] = per-group (mean, rstd)
                gvals = small.tile([16, 2], F32, tag="gvals", name="gvals")
                nc.vector.tensor_copy(gvals[:, 0:1], mall[:, 0:1])
                nc.vector.tensor_tensor(gvals[:, 1:2], r0[:], t3[:], op=OP.mult)
                for ct in range(2):
                    cv = ps_gn.tile([128, 2], F32, tag="cv", name="cv")
                    nc.tensor.matmul(
                        cv[:], gbc_sb[:, ct * 128 : (ct + 1) * 128], gvals[:],
                        start=True, stop=True,
                    )
                    scale_t = small.tile([128, 1], F32, tag="scale", name="scale")
                    nc.vector.tensor_tensor(scale_t[:], gnp_sb[ct][:, 0:1], cv[:, 1:2], op=OP.mult)
                    tb = small.tile([128, 1], F32, tag="tb", name="tb")
                    nc.vector.tensor_tensor(tb[:], cv[:, 0:1], scale_t[:], op=OP.mult)
                    bias_t = small.tile([128, 1], F32, tag="bias", name="bias")
                    nc.vector.tensor_tensor(bias_t[:], gnp_sb[ct][:, 1:2], tb[:], op=OP.subtract)
                    nc.vector.tensor_scalar(
                        xn_sb[ct][:], x_sb[ct][:], scale_t[:], bias_t[:],
                        op0=OP.mult, op1=OP.add,
                    )

            # ---- QKV projections ----
            q_sb = big.tile([128, HW], mm_dt, tag="q", name="q")
            k_sb = big.tile([128, HW], mm_dt, tag="k", name="k")
            vt_sb = []
            for h in range(2):
                t = big.tile([128, NJ * (HD + 1)], mm_dt, tag=f"vt{h}", name=f"vt{h}")
                t3 = t[:].rearrange("p (j c) -> p j c", c=HD + 1)
                # fill the per-chunk "ones" column (denominator accumulator rows);
                # memset can't write f32r, so use (x*0 + 1) via tensor_scalar
                nc.vector.tensor_scalar(
                    t3[:, :, HD : HD + 1], x_sb[0][:, 0:NJ], 0.0, 1.0,
                    op0=OP.mult, op1=OP.add,
                )
                vt_sb.append(t)

            with tc.tile_pool(name="ps_qkv", bufs=2, space=bass.MemorySpace.PSUM) as ps_qkv:
                # v^T first: its DVE copies then overlap the q/k matmuls, and
                # q/k (which attention needs first) are ready right at the boundary
                for half in range(2):
                    vps = ps_qkv.tile([128, HALF], F32, tag="qkv", name="qkv")
                    for j9 in range(9):
                        jc = half * 9 + j9
                        for kc in range(2):
                            nc.tensor.matmul(
                                vps[:, j9 * 128 : (j9 + 1) * 128],
                                xn_sb[kc][:, jc * JC : (jc + 1) * JC],
                                w_sb["wv", kc][:],
                                start=(kc == 0), stop=(kc == 1),
                            )
                    vps3 = vps[:].rearrange("p (j c) -> p j c", c=128)
                    for h in range(2):
                        dst3 = vt_sb[h][:].rearrange("p (j c) -> p j c", c=HD + 1)
                        nc.vector.tensor_copy(
                            dst3[:, half * 9 : (half + 1) * 9, 0:HD],
                            vps3[:, :, h * HD : (h + 1) * HD],
                        )
                for dst, wname in ((q_sb, "wq"), (k_sb, "wk")):
                    for half in range(2):
                        ps = ps_qkv.tile([128, HALF], F32, tag="qkv", name="qkv")
                        for kc in range(2):
                            for n0, n1 in _nchunks(HALF):
                                nc.tensor.matmul(
                                    ps[:, n0:n1],
                                    cast(w_sb[wname, kc][:]),
                                    cast(xn_sb[kc][:, half * HALF + n0 : half * HALF + n1]),
                                    start=(kc == 0), stop=(kc == 1),
                                )
                        nc.vector.tensor_copy(dst[:, half * HALF : (half + 1) * HALF], ps[:])

            # ---- attention ----
            headout = big.tile([128, HW], mm_dt, tag="headout", name="headout")
            with tc.tile_pool(name="ps_att", bufs=1, space=bass.MemorySpace.PSUM) as ps_att:
                for i0, i1 in IBLKS:
                    blk = i1 - i0
                    # h0/h1 S^T outputs must land in DIFFERENT psum banks: concurrent
                    # row-tiled matmuls writing the same bank crash the device.
                    salign = ((blk + 511) // 512) * 512
                    u = [ps_att.tile([HD + 1, blk], F32, tag=f"u{h}", name=f"u{h}", bufs=2) for h in range(2)]
                    def emit_s(jc):
                        st = ps_att.tile([128, 2 * salign], F32, tag="s", name="s", bufs=2)
                        for h in range(2):
                            lhsT = k_sb[h * HD : (h + 1) * HD, jc * JC : (jc + 1) * JC]
                            for n0, n1 in _nchunks(blk, 512):
                                nc.tensor.matmul(
                                    st[:, h * salign + n0 : h * salign + n1],
                                    cast(lhsT),
                                    cast(q_sb[h * HD : (h + 1) * HD, i0 + n0 : i0 + n1]),
                                    start=True, stop=True,
                                )
                        pt = ptp.tile([128, 2 * blk], mm_dt, tag="pt", name="pt")
                        if blk == salign:
                            nc.scalar.activation(pt[:], st[:], AF.Exp, scale=1.0 / 16.0)
                        else:
                            for h in range(2):
                                nc.scalar.activation(
                                    pt[:, h * blk : (h + 1) * blk],
                                    st[:, h * salign : h * salign + blk],
                                    AF.Exp, scale=1.0 / 16.0,
                                )
                        return pt

                    def emit_pv(jc, pt):
                        for h in range(2):
                            lhsT = vt_sb[h][:, jc * (HD + 1) : (jc + 1) * (HD + 1)]
                            for n0, n1 in _nchunks(blk, 512):
                                nc.tensor.matmul(
                                    u[h][:, n0:n1],
                                    cast(lhsT),
                                    cast(pt[:, h * blk + n0 : h * blk + n1]),
                                    start=(jc == 0), stop=(jc == NJ - 1),
                                )

                    # software-pipeline by one stage: PE computes S(jc+1) while
                    # ACT exponentiates S(jc), so the PE stream never blocks on exp
                    prev_pt = emit_s(0)
                    for jc in range(1, NJ):
                        pt = emit_s(jc)
                        emit_pv(jc - 1, prev_pt)
                        prev_pt = pt
                    emit_pv(NJ - 1, prev_pt)
                    for h in range(2):
                        dn = small.tile([1, blk], F32, tag="dn", name="dn")
                        nc.vector.tensor_copy(dn[:], u[h][HD : HD + 1, :])
                        rcp = small.tile([1, blk], F32, tag="rcp", name="rcp")
                        scr = small.tile([1, blk], F32, tag="scr", name="scr")
                        nc.vector.reciprocal_approx_accurate(rcp[:], dn[:], scr[:])
                        rb = small.tile([HD, blk], F32, tag="rb", name="rb")
                        nc.gpsimd.partition_broadcast(rb[:], rcp[:])
                        nc.vector.tensor_tensor(
                            headout[h * HD : (h + 1) * HD, i0:i1],
                            u[h][0:HD, :], rb[:], op=OP.mult,
                        )

            # ---- output projection + residual ----
            with tc.tile_pool(name="ps_out", bufs=2, space=bass.MemorySpace.PSUM) as ps_out:
                for mt in range(2):
                    for half in range(2):
                        yp = ps_out.tile([128, HALF], F32, tag="yp", name="yp")
                        for n0, n1 in _nchunks(HALF):
                            nc.tensor.matmul(
                                yp[:, n0:n1],
                                cast(wo_sb[:, mt * 128 : (mt + 1) * 128]),
                                cast(headout[:, half * HALF + n0 : half * HALF + n1]),
                                start=True, stop=True,
                            )
                        yo = small.tile([128, HALF], F32, tag="yo", name="yo")
                        nc.vector.tensor_tensor(
                            yo[:], yp[:],
                            res_sb[mt][:, half * HALF : (half + 1) * HALF], op=OP.add,
                        )
                        nc.sync.dma_start(
                            y_d[mt * 128 : (mt + 1) * 128, half * HALF : (half + 1) * HALF],
                            yo[:],
                        )

    nc.compile()
    return nc


def _consts():
    # gind[:, 0:16]: tile-0 channel -> group one-hot; [:, 16:32]: tile-1 channel -> group
    gind = np.zeros((128, 32), np.float32)
    for c in range(128):
        gind[c, c // GC] = 1.0
        gind[c, 16 + 8 + c // GC] = 1.0
    gbc = np.zeros((16, C), np.float32)
    for c in range(C):
        gbc[c // GC, c] = 1.0
    return gind, gbc


def make_in_maps(x, gn_weight, gn_bias, qkv_w, out_w, out_b):
    x = np.asarray(x, np.float32)
    qkv_w = np.asarray(qkv_w, np.float32)
    out_w = np.asarray(out_w, np.float32)
    out_b = np.asarray(out_b, np.float32)
    gn_weight = np.asarray(gn_weight, np.float32)
    gn_bias = np.asarray(gn_bias, np.float32)
    xr = np.ascontiguousarray(x.reshape(B, C, HW))
    gind, gbc = _consts()
    gnp = np.ascontiguousarray(np.stack([gn_weight, gn_bias], axis=1))
    in_maps = []
    for core in range(NCORES):
        b, hp = divmod(core, 2)
        heads = (2 * hp, 2 * hp + 1)
        qs = np.concatenate([qkv_w[n * 192 : n * 192 + 64] for n in heads], 0)
        ks = np.concatenate([qkv_w[n * 192 + 64 : n * 192 + 128] for n in heads], 0)
        vs = np.concatenate([qkv_w[n * 192 + 128 : n * 192 + 192] for n in heads], 0)
        res = xr[b] + out_b[:, None] if hp == 0 else np.zeros_like(xr[b])
        in_maps.append({
            "x": xr[b],
            "res": np.ascontiguousarray(res, np.float32),
            "wq": np.ascontiguousarray(qs.T),
            "wk": np.ascontiguousarray(ks.T),
            "wv": np.ascontiguousarray(vs.T),
            "wo": np.ascontiguousarray(out_w[:, hp * 128 : (hp + 1) * 128].T),
            "gnp": gnp,
            "gind": gind,
            "gbc": gbc,
        })
    return in_maps


_NC_CACHE = {}


def get_nc(mm_dt=F32R):
    key = str(mm_dt)
    if key not in _NC_CACHE:
        _NC_CACHE[key] = _build(mm_dt)
    return _NC_CACHE[key]


def kernel(x, gn_weight, gn_bias, qkv_w, out_w, out_b):
    nc = get_nc(BF16)
    in_maps = make_in_maps(x, gn_weight, gn_bias, qkv_w, out_w, out_b)
    res = bass_utils.run_bass_kernel_spmd(nc, in_maps, core_ids=list(range(NCORES)))
    y = np.empty((B, C, HW), np.float32)
    for b in range(B):
        y[b] = res.results[2 * b]["y"] + res.results[2 * b + 1]["y"]
    return y.reshape(B, C, H, W)



# revision 3
# speedup vs baseline: 1.0308x; 1.0308x over previous
"""Trainium2 Bass kernel for spatial attention (GroupNorm + QKV + softmax attention
+ output projection + residual), distributed over 8 NeuronCores.

Sharding: core = 2*b + hp handles image b (of 4) and head pair hp (heads 2hp, 2hp+1).
Each core computes GroupNorm(x[b]), its heads' q/k/v, full spatial attention for its
two heads, and a partial output projection (out_w columns for its heads). Core pairs'
partial outputs are summed on the host (hp==0 core carries the residual + bias).

Perf notes (v2):
- Scores stay bf16, row-tiled so both heads' S^T matmuls run concurrently.
- softmax exp is computed 2-of-3 on the DVE via a Schraudolph bit-trick directly
  into fp8e4m3 bits (uint8 = trunc(s*A + B) reinterpreted as fp8), 1-of-3 on ACT
  (fp8 output) — splitting the 10.6M-element exp load across engines.
- PV runs as fp8 DoubleRow matmuls (two key-chunks = K=256 per instruction),
  halving PV stream time; v^T keeps a 65th all-ones column so the softmax
  denominator accumulates in the same matmul.
- Output projection + residual + store are pipelined per-iblk (lag 1) so the PE
  stays dense through the end of the kernel (HAM stays warm).
"""

import math

import numpy as np

import concourse.bass as bass
import concourse.bacc as bacc
import concourse.tile as tile
from concourse import mybir
from concourse import bass_utils
from concourse.alu_op_type import AluOpType

B, C, H, W = 4, 256, 48, 48
HW = H * W  # 2304
NH, HD = 4, 64
G, GC = 16, 16  # 16 groups x 16 channels
EPS = 1e-5
NCORES = 8
JC = 128  # j (key spatial) chunk
NJ = HW // JC  # 18
NJP = NJ // 2  # 9 key-chunk pairs (DoubleRow K=256)
IBLKS = [(0, 512), (512, 1024), (1024, 1536), (1536, 2048), (2048, 2304)]
HALF = HW // 2  # 1152
PVM = HD + 1  # 65: 64 v channels + denominator ones row
VST = 80  # fp8 v^T subtile stride (16-byte aligned, >= PVM)
SALIGN = 512

F32 = mybir.dt.float32
BF16 = mybir.dt.bfloat16
F8 = mybir.dt.float8e4
U8 = mybir.dt.uint8
AX = mybir.AxisListType.X
AF = mybir.ActivationFunctionType
OP = AluOpType
PM = mybir.MatmulPerfMode

# Schraudolph exp into fp8e4m3 bit space: bits = trunc(s*EXPA + EXPB),
# value(bits) ~= exp(s/16).  EXPB tuned numerically for minimax rel err (~7%)
# assuming truncation on the DVE float->uint8 convert.
EXPA = 8.0 * math.log2(math.e) / 16.0
EXPB = 56.13
ACT_EVERY = 3  # key chunks with jc % ACT_EVERY == ACT_EVERY-1 exp on ACT


def _nchunks(size, step=512):
    # PSUM-bank-aligned chunks: a matmul output may not cross a 512-fp32 bank boundary
    return [(a, min(a + step, size)) for a in range(0, size, step)]


def _build(mm_dt=BF16):
    nc = bacc.Bacc("TRN2", target_bir_lowering=False, debug=False, enable_asserts=False)

    x_d = nc.dram_tensor("x", [C, HW], F32, kind="ExternalInput").ap()
    res_d = nc.dram_tensor("res", [C, HW], F32, kind="ExternalInput").ap()
    wq_d = nc.dram_tensor("wq", [C, 2 * HD], F32, kind="ExternalInput").ap()
    wk_d = nc.dram_tensor("wk", [C, 2 * HD], F32, kind="ExternalInput").ap()
    wv_d = nc.dram_tensor("wv", [C, 2 * HD], F32, kind="ExternalInput").ap()
    wo_d = nc.dram_tensor("wo", [2 * HD, C], F32, kind="ExternalInput").ap()
    gnp_d = nc.dram_tensor("gnp", [C, 2], F32, kind="ExternalInput").ap()
    gind_d = nc.dram_tensor("gind", [128, 32], F32, kind="ExternalInput").ap()
    gbc_d = nc.dram_tensor("gbc", [16, C], F32, kind="ExternalInput").ap()
    y_d = nc.dram_tensor("y", [C, HW], F32, kind="ExternalOutput").ap()

    with tile.TileContext(nc) as tc:
        with (
            tc.tile_pool(name="consts", bufs=1) as consts,
            tc.tile_pool(name="big", bufs=1) as big,
            tc.tile_pool(name="small", bufs=4) as small,
            tc.tile_pool(name="pt", bufs=3) as ptp,
        ):
            # ---- input x first (GN stats are the critical path) ----
            x_sb, xn_sb, res_sb = [], [], []
            for ct in range(2):
                t = big.tile([128, HW], F32, tag=f"x{ct}", name=f"x{ct}")
                for a, b_ in _nchunks(HW, 1152):
                    nc.sync.dma_start(t[:, a:b_], x_d[ct * 128 : (ct + 1) * 128, a:b_])
                x_sb.append(t)
                xn_sb.append(big.tile([128, HW], mm_dt, tag=f"xn{ct}", name=f"xn{ct}"))

            # ---- constant / weight loads ----
            gind_sb = consts.tile([128, 32], F32, tag="gind", name="gind")
            nc.sync.dma_start(gind_sb[:], gind_d[:])
            gbc_sb = consts.tile([16, C], F32, tag="gbc", name="gbc")
            nc.sync.dma_start(gbc_sb[:], gbc_d[:])
            gnp_sb = []
            for ct in range(2):
                t = consts.tile([128, 2], F32, tag=f"gnp{ct}", name=f"gnp{ct}")
                nc.sync.dma_start(t[:], gnp_d[ct * 128 : (ct + 1) * 128, :])
                gnp_sb.append(t)
            w_sb = {}
            for name, d in (("wq", wq_d), ("wk", wk_d), ("wv", wv_d)):
                for kc in range(2):
                    tf = consts.tile([128, 2 * HD], F32, tag=f"{name}{kc}f", name=f"{name}{kc}f")
                    nc.sync.dma_start(tf[:], d[kc * 128 : (kc + 1) * 128, :])
                    t = consts.tile([128, 2 * HD], mm_dt, tag=f"{name}{kc}", name=f"{name}{kc}")
                    nc.vector.tensor_copy(t[:], tf[:])
                    w_sb[name, kc] = t
            wof = consts.tile([128, C], F32, tag="wof", name="wof")
            nc.sync.dma_start(wof[:], wo_d[:])
            wo_sb = consts.tile([128, C], mm_dt, tag="wo", name="wo")
            nc.vector.tensor_copy(wo_sb[:], wof[:])
            for ct in range(2):
                r = big.tile([128, HW], F32, tag=f"res{ct}", name=f"res{ct}")
                nc.sync.dma_start(r[:], res_d[ct * 128 : (ct + 1) * 128, :])
                res_sb.append(r)

            # ---- GroupNorm ----
            # per-channel sums / sum-of-squares -> stats[:, (s0,q0,s1,q1)]
            stats = small.tile([128, 4], F32, tag="stats", name="stats")
            for ct in range(2):
                nc.vector.reduce_sum(stats[:, 2 * ct : 2 * ct + 1], x_sb[ct][:], axis=AX)
                nc.vector.scalar_tensor_tensor(
                    xn_sb[ct][:], x_sb[ct][:], 1.0, x_sb[ct][:],
                    op0=OP.mult, op1=OP.mult,
                    accum_out=stats[:, 2 * ct + 1 : 2 * ct + 2],
                )
            with tc.tile_pool(name="ps_gn", bufs=2, space=bass.MemorySpace.PSUM) as ps_gn:
                # accumulate both channel tiles' per-group (sum, sumsq) into [16, 2]
                g_ps = ps_gn.tile([16, 2], F32, tag="g", name="g")
                nc.tensor.matmul(g_ps[:], gind_sb[:, 0:16], stats[:, 0:2], start=True, stop=False)
                nc.tensor.matmul(g_ps[:], gind_sb[:, 16:32], stats[:, 2:4], start=False, stop=True)
                mall = small.tile([16, 2], F32, tag="mall", name="mall")
                nc.vector.tensor_scalar_mul(mall[:], g_ps[:], 1.0 / (GC * HW))
                msq = small.tile([16, 1], F32, tag="msq", name="msq")
                nc.vector.tensor_tensor(msq[:], mall[:, 0:1], mall[:, 0:1], op=OP.mult)
                ve = small.tile([16, 1], F32, tag="ve", name="ve")
                nc.vector.tensor_tensor(ve[:], mall[:, 1:2], msq[:], op=OP.subtract)
                ve2 = small.tile([16, 1], F32, tag="ve2", name="ve2")
                nc.vector.tensor_scalar_add(ve2[:], ve[:], EPS)
                # rstd = exp(-0.5 * ln(v)) — keeps ACT in the natural_log_exp
                # table set (shared with attention's Exp: one table load)
                lg = small.tile([16, 1], F32, tag="lg", name="lg")
                nc.scalar.activation(lg[:], ve2[:], AF.Ln)
                # gvals [16, 2] = per-group (mean, rstd)
                gvals = small.tile([16, 2], F32, tag="gvals", name="gvals")
                nc.vector.tensor_copy(gvals[:, 0:1], mall[:, 0:1])
                nc.scalar.activation(gvals[:, 1:2], lg[:], AF.Exp, scale=-0.5)
                for ct in range(2):
                    cv = ps_gn.tile([128, 2], F32, tag="cv", name="cv")
                    nc.tensor.matmul(
                        cv[:], gbc_sb[:, ct * 128 : (ct + 1) * 128], gvals[:],
                        start=True, stop=True,
                    )
                    scale_t = small.tile([128, 1], F32, tag="scale", name="scale")
                    nc.vector.tensor_tensor(scale_t[:], gnp_sb[ct][:, 0:1], cv[:, 1:2], op=OP.mult)
                    tb = small.tile([128, 1], F32, tag="tb", name="tb")
                    nc.vector.tensor_tensor(tb[:], cv[:, 0:1], scale_t[:], op=OP.mult)
                    bias_t = small.tile([128, 1], F32, tag="bias", name="bias")
                    nc.vector.tensor_tensor(bias_t[:], gnp_sb[ct][:, 1:2], tb[:], op=OP.subtract)
                    nc.vector.tensor_scalar(
                        xn_sb[ct][:], x_sb[ct][:], scale_t[:], bias_t[:],
                        op0=OP.mult, op1=OP.add,
                    )

            # ---- QKV projections ----
            q_sb = big.tile([128, HW], mm_dt, tag="q", name="q")
            k_sb = big.tile([128, HW], mm_dt, tag="k", name="k")
            # v^T in fp8, laid out [128 spatial, NJ subtiles of VST]: subtile jc
            # holds chunk jc's [64 v-channels + ones column(s)].  Pair 2p,2p+1
            # forms the DoubleRow K=256 stationary operand.
            vt_sb, vt_v = [], []
            for h in range(2):
                t = big.tile([128, NJ * VST], F8, tag=f"vt{h}", name=f"vt{h}")
                nc.gpsimd.memset(t[:], 1.0)  # ones columns (and padding)
                vt_sb.append(t)
                vt_v.append(t[:].rearrange("p (j c) -> p j c", c=VST))

            with tc.tile_pool(name="ps_qkv", bufs=2, space=bass.MemorySpace.PSUM) as ps_qkv:
                # v^T first: its DVE casts then overlap the q/k matmuls, and
                # q/k (which attention needs first) are ready right at the boundary
                for half in range(2):
                    vps = ps_qkv.tile([128, HALF], F32, tag="qkv", name="qkv")
                    for j9 in range(9):
                        jc = half * 9 + j9
                        for kc in range(2):
                            nc.tensor.matmul(
                                vps[:, j9 * 128 : (j9 + 1) * 128],
                                xn_sb[kc][:, jc * JC : (jc + 1) * JC],
                                w_sb["wv", kc][:],
                                start=(kc == 0), stop=(kc == 1),
                            )
                    vps3 = vps[:].rearrange("p (j c) -> p j c", c=128)
                    for h in range(2):
                        nc.vector.tensor_copy(
                            vt_v[h][:, half * 9 : (half + 1) * 9, 0:HD],
                            vps3[:, :, h * HD : (h + 1) * HD],
                        )
                for dst, wname in ((k_sb, "wk"), (q_sb, "wq")):
                    for half in range(2):
                        ps = ps_qkv.tile([128, HALF], F32, tag="qkv", name="qkv")
                        for kc in range(2):
                            for n0, n1 in _nchunks(HALF):
                                nc.tensor.matmul(
                                    ps[:, n0:n1],
                                    w_sb[wname, kc][:],
                                    xn_sb[kc][:, half * HALF + n0 : half * HALF + n1],
                                    start=(kc == 0), stop=(kc == 1),
                                )
                        nc.vector.tensor_copy(dst[:, half * HALF : (half + 1) * HALF], ps[:])

            # ---- attention + pipelined output projection ----
            with (
                tc.tile_pool(name="ps_att", bufs=1, space=bass.MemorySpace.PSUM) as ps_att,
                tc.tile_pool(name="ps_out", bufs=2, space=bass.MemorySpace.PSUM) as ps_out,
            ):
                pending = None  # closure emitting previous iblk's output projection

                def emit_outproj(i0, i1, ho):
                    blk = i1 - i0
                    for mt in range(2):
                        yp = ps_out.tile([128, blk], F32, tag="yp", name="yp")
                        nc.tensor.matmul(
                            yp[:], wo_sb[:, mt * 128 : (mt + 1) * 128], ho[:],
                            start=True, stop=True,
                        )
                        yo = small.tile([128, blk], F32, tag="yo", name="yo")
                        nc.vector.tensor_tensor(
                            yo[:], yp[:], res_sb[mt][:, i0:i1], op=OP.add,
                        )
                        nc.sync.dma_start(y_d[mt * 128 : (mt + 1) * 128, i0:i1], yo[:])

                for i0, i1 in IBLKS:
                    blk = i1 - i0
                    # u[h]: [65, blk] accumulator (64 channels + denominator row)
                    u = [
                        ps_att.tile([PVM, SALIGN], F32, tag=f"u{h}", name=f"u{h}", bufs=1)
                        for h in range(2)
                    ]

                    def emit_s(jc):
                        # S^T chunk for both heads, row-tiled (concurrent on PE).
                        # h0/h1 outputs land in different psum banks.
                        st = ps_att.tile([128, 2 * SALIGN], F32, tag="s", name="s", bufs=2)
                        for h in range(2):
                            nc.tensor.matmul(
                                st[:, h * SALIGN : h * SALIGN + blk],
                                k_sb[h * HD : (h + 1) * HD, jc * JC : (jc + 1) * JC],
                                q_sb[h * HD : (h + 1) * HD, i0:i1],
                                start=True, stop=True,
                            )
                        return st

                    def emit_exp(jc, st, pair_v, s):
                        # pt8[:, h, s, :] = exp(st[:, h, :]/16) as fp8e4m3
                        src = st[:].rearrange("p (h x) -> p h x", h=2)[:, :, 0:blk]
                        dst = pair_v[:, :, s, 0:blk]
                        if jc % ACT_EVERY == ACT_EVERY - 1:
                            nc.scalar.activation(dst, src, AF.Exp, scale=1.0 / 16.0)
                        else:
                            nc.vector.tensor_scalar(
                                dst.bitcast(U8), src, EXPA, EXPB,
                                op0=OP.mult, op1=OP.add,
                            )

                    def emit_pv(pp, pair_v):
                        # DoubleRow fp8: contracts both chunks of the pair (K=256)
                        for h in range(2):
                            nc.tensor.matmul(
                                u[h][:, 0:blk],
                                vt_v[h][:, 2 * pp : 2 * pp + 2, 0:PVM],
                                pair_v[:, h, :, 0:blk],
                                start=(pp == 0), stop=(pp == NJP - 1),
                                perf_mode=PM.DoubleRow,
                            )

                    prev_pair = None
                    for pp in range(NJP):
                        pt = ptp.tile([128, 4 * blk], F8, tag="pt8", name="pt8")
                        pair_v = pt[:].rearrange("p (h s x) -> p h s x", h=2, s=2)
                        for s in range(2):
                            jc = 2 * pp + s
                            st = emit_s(jc)
                            emit_exp(jc, st, pair_v, s)
                        if pp == 1 and pending is not None:
                            pending()
                            pending = None
                        if pp > 0:
                            emit_pv(pp - 1, prev_pair)
                        prev_pair = pair_v
                    emit_pv(NJP - 1, prev_pair)

                    ho = ptp.tile([128, blk], mm_dt, tag="ho", name="ho", bufs=2)
                    for h in range(2):
                        dn = small.tile([1, blk], F32, tag="dn", name="dn")
                        nc.vector.tensor_copy(dn[:], u[h][HD : HD + 1, 0:blk])
                        rcp = small.tile([1, blk], F32, tag="rcp", name="rcp")
                        scr = small.tile([1, blk], F32, tag="scr", name="scr")
                        nc.vector.reciprocal_approx_accurate(rcp[:], dn[:], scr[:])
                        rb = small.tile([HD, blk], F32, tag="rb", name="rb")
                        nc.gpsimd.partition_broadcast(rb[:], rcp[:])
                        nc.vector.tensor_tensor(
                            ho[h * HD : (h + 1) * HD, :],
                            u[h][0:HD, 0:blk], rb[:], op=OP.mult,
                        )
                    pending = (lambda a=i0, b_=i1, t=ho: emit_outproj(a, b_, t))
                pending()

    nc.compile()
    return nc


def _consts():
    # gind[:, 0:16]: tile-0 channel -> group one-hot; [:, 16:32]: tile-1 channel -> group
    gind = np.zeros((128, 32), np.float32)
    for c in range(128):
        gind[c, c // GC] = 1.0
        gind[c, 16 + 8 + c // GC] = 1.0
    gbc = np.zeros((16, C), np.float32)
    for c in range(C):
        gbc[c // GC, c] = 1.0
    return gind, gbc


def make_in_maps(x, gn_weight, gn_bias, qkv_w, out_w, out_b):
    x = np.asarray(x, np.float32)
    qkv_w = np.asarray(qkv_w, np.float32)
    out_w = np.asarray(out_w, np.float32)
    out_b = np.asarray(out_b, np.float32)
    gn_weight = np.asarray(gn_weight, np.float32)
    gn_bias = np.asarray(gn_bias, np.float32)
    xr = np.ascontiguousarray(x.reshape(B, C, HW))
    gind, gbc = _consts()
    gnp = np.ascontiguousarray(np.stack([gn_weight, gn_bias], axis=1))
    in_maps = []
    for core in range(NCORES):
        b, hp = divmod(core, 2)
        heads = (2 * hp, 2 * hp + 1)
        qs = np.concatenate([qkv_w[n * 192 : n * 192 + 64] for n in heads], 0)
        ks = np.concatenate([qkv_w[n * 192 + 64 : n * 192 + 128] for n in heads], 0)
        vs = np.concatenate([qkv_w[n * 192 + 128 : n * 192 + 192] for n in heads], 0)
        res = xr[b] + out_b[:, None] if hp == 0 else np.zeros_like(xr[b])
        in_maps.append({
            "x": xr[b],
            "res": np.ascontiguousarray(res, np.float32),
            "wq": np.ascontiguousarray(qs.T),
            "wk": np.ascontiguousarray(ks.T),
            "wv": np.ascontiguousarray(vs.T),
            "wo": np.ascontiguousarray(out_w[:, hp * 128 : (hp + 1) * 128].T),
            "gnp": gnp,
            "gind": gind,
            "gbc": gbc,
        })
    return in_maps


_NC_CACHE = {}


def get_nc(mm_dt=BF16):
    key = str(mm_dt)
    if key not in _NC_CACHE:
        _NC_CACHE[key] = _build(mm_dt)
    return _NC_CACHE[key]


def kernel(x, gn_weight, gn_bias, qkv_w, out_w, out_b):
    nc = get_nc(BF16)
    in_maps = make_in_maps(x, gn_weight, gn_bias, qkv_w, out_w, out_b)
    res = bass_utils.run_bass_kernel_spmd(nc, in_maps, core_ids=list(range(NCORES)))
    y = np.empty((B, C, HW), np.float32)
    for b in range(B):
        y[b] = res.results[2 * b]["y"] + res.results[2 * b + 1]["y"]
    return y.reshape(B, C, H, W)
